# revision 39
# baseline (speedup 1.0000x reference)
"""PPO-style loss kernel for Trainium2, data-parallel over the env axis B.

Contract: kernel(**inputs) takes the full unsharded inputs (numpy or jax
arrays) keyed as in setup_inputs() and returns the full scalar loss
(np.float32). Internally shards B=1024 across 8 NeuronCores (128 envs
each), runs one SPMD Bass program per core, and combines the per-core
partial sums on the host.

Structure (per core, 128 envs):
  - obs [129,128,256] arrives bf16, is DMA-xbar-transposed per row-chunk
    into feature-major tiles, normalized ((x*s+b) then clip +-5) on DVE and
    cast to fp8e4m3.
  - Both MLPs run feature-major in fp8 with DoubleRow matmuls (2 k-rows per
    PE cell); L1 computes policy+value layer-1 in one pass over a fused
    [256,1024] weight. SiLU + psum evacuation fused on the scalar engine.
  - Policy logits are PE-transposed per timestep into an [env, t, 64]
    resident tile; values land via small PE transposes into [env, t+1].
  - The distribution math (tanh-Normal log-probs, entropy) runs batch-major
    as a tail gated behind the last SiLU evacuation so its Exp/Ln chain
    never interleaves with SiLU (different ACT LUT tables; each interleave
    costs two 1.28us table loads).
  - GAE is one DVE tensor_tensor_scan over reversed [env, t] tiles.
  - rho = exp(target_lp - behaviour_lp): the tanh log-det-jacobian and
    0.5*log(2pi) terms cancel and are never computed. exp overflow lanes
    (d > 88.72, where fp32 gives inf and the reference loss goes nan via
    inf*0) are counted on-chip and the ieee result is reinstated on host.
"""

import os
import sys

for _p in ("/opt/trn_rl_repo", "/root/.axon_site/_ro/trn_rl_repo"):
    if os.path.isdir(_p) and _p not in sys.path:
        sys.path.insert(0, _p)

import numpy as np
import ml_dtypes

import concourse.bass as bass
import concourse.tile as tile
from concourse import bacc, mybir
from concourse import bass_utils
from concourse.masks import make_identity

F32 = mybir.dt.float32
BF16 = mybir.dt.bfloat16
F8 = mybir.dt.float8e4
AX = mybir.AxisListType
OP = mybir.AluOpType
AF = mybir.ActivationFunctionType
DR = mybir.MatmulPerfMode.DoubleRow

T, B, OBS, ACTD, H = 128, 1024, 256, 32, 512
NCORES = 8
BL = B // NCORES                 # 128 envs per core
ROWS = (T + 1) * BL              # 16512 rows through the value net
PROWS = T * BL                   # 16384 rows through the policy net
NS = 512                         # row-chunk size
NCH = (ROWS + NS - 1) // NS      # 33 chunks (last one has 128 rows)
PCH = PROWS // NS                # 32 policy chunks
GT = 32                          # timesteps per distribution group
NG = T // GT                     # 4 groups

GAMMA = 0.97
LAMBDA = 0.95
CLIPEPS = 0.3
REW_SCALE = 0.1
ENT_COST = 0.01
LOG_2PI_HALF = 0.5 * float(np.log(2.0 * np.pi))
LOG2 = float(np.log(2.0))
ENT_CONST = ACTD * (0.5 + LOG_2PI_HALF + 2.0 * LOG2)
EXP_OVF = 88.72283                     # exp(x) == inf in fp32 above this

_BF = ml_dtypes.bfloat16
_F8 = mybir.dt.np(F8)


def _rev(ap):
    """Reverse the (single) free dim of a 2D [P, F] AP view."""
    (ps, pc), (fs, fc) = ap.ap[0], ap.ap[1]
    return bass.AP(
        tensor=ap.tensor,
        offset=ap.offset + (fc - 1) * fs,
        ap=[[ps, pc], [-fs, fc]],
    )


def emit(tc, io):
    nc = tc.nc
    from contextlib import ExitStack

    ctx = ExitStack()
    with ctx:
        consts = ctx.enter_context(tc.tile_pool(name="consts", bufs=1))
        chpool = ctx.enter_context(tc.tile_pool(name="ch", bufs=2))
        gio = ctx.enter_context(tc.tile_pool(name="gio", bufs=2))
        scr2 = ctx.enter_context(tc.tile_pool(name="scr2", bufs=2))
        scr1 = ctx.enter_context(tc.tile_pool(name="scr1", bufs=1))
        scrp = ctx.enter_context(tc.tile_pool(name="scrp", bufs=2))
        gae = ctx.enter_context(tc.tile_pool(name="gae", bufs=1))
        pmm = ctx.enter_context(tc.tile_pool(name="pmm", bufs=3, space="PSUM"))
        pts = ctx.enter_context(tc.tile_pool(name="pts", bufs=2, space="PSUM"))

        # ---- constants / weights into SBUF ----
        w0_sb = consts.tile([128, 2, 2 * H], F8)          # [ki, ko, m]
        nc.sync.dma_start(out=w0_sb, in_=io["w0"].rearrange("ko ki m -> ki ko m"))
        w1p_sb = consts.tile([128, 2, 2, H], F8)          # [ki, kb, ko, m]
        nc.sync.dma_start(out=w1p_sb,
                          in_=io["w1p"].rearrange("kb ki ko m -> ki kb ko m"))
        w1v_sb = consts.tile([128, 2, 2, H], F8)
        nc.sync.dma_start(out=w1v_sb,
                          in_=io["w1v"].rearrange("kb ki ko m -> ki kb ko m"))
        w2p_sb = consts.tile([128, 2, 2, 2 * ACTD], F8)
        nc.sync.dma_start(out=w2p_sb,
                          in_=io["w2p"].rearrange("kb ki ko m -> ki kb ko m"))
        # value head padded to M=16: DoubleRow ldweights needs the k-pair
        # stride (= M elements) to be 16-byte aligned
        w2v_sb = consts.tile([128, 2, 2, 16], F8)
        nc.sync.dma_start(out=w2v_sb,
                          in_=io["w2v"].rearrange("kb ki ko m -> ki kb ko m"))
        s_sb = consts.tile([128, 2], F32)
        nc.sync.dma_start(out=s_sb, in_=io["scl"])
        nms_sb = consts.tile([128, 2], F32)
        nc.sync.dma_start(out=nms_sb, in_=io["nms"])
        pb2_sb = consts.tile([64, 1], F32)
        nc.sync.dma_start(out=pb2_sb, in_=io["pb2"])
        vb2_sb = consts.tile([1, 1], F32)
        nc.sync.dma_start(out=vb2_sb, in_=io["vb2"])

        id_f = consts.tile([128, 128], F32)
        make_identity(nc, id_f)
        ones_sb = consts.tile([128, 1], F32)
        nc.vector.memset(ones_sb, 1.0)
        c_eps = consts.tile([128, 1], F32)          # 0.001 bias for Ln(sp+eps)
        nc.vector.memset(c_eps, 0.001)

        # persistent accumulators ([env, ...] layouts)
        pol_bt = consts.tile([128, T, 2 * ACTD], F32)    # policy logits
        vals = consts.tile([128, T + 1], F32)            # values + bootstrap
        dsum = consts.tile([128, T], F32)                # target_lp - behaviour_lp
        entsum = consts.tile([128, T], F32)              # entropy partials
        sums = consts.tile([128, 8], F32)

        obs_flat = io["obs"].flatten_outer_dims()        # [16512, 256]
        last_silu = None
        QR = 4 * NS                                      # xbar piece: 4 chunks

        # ---- main row-chunk loop ----
        for c in range(NCH):
            ns = min(NS, ROWS - c * NS)
            R = slice(c * NS, c * NS + ns)

            # obs: xbar-transpose to feature-major bf16 in 2048-row pieces,
            # then per chunk z = clip(x*s + nms, +-5) and cast to fp8 on DVE
            if c % 4 == 0:
                qr = min(QR, ROWS - c * NS)
                obsQ = chpool.tile([128, 2, QR], BF16, tag="obsQ")
                # one xbar DMA per (chunk, kc) sub-range: the first chunk's
                # normalize then only waits ~0.9us for its own 512 rows
                # instead of the full 2048-row piece transfer
                for cs in range(0, qr, NS):
                    cn = min(NS, qr - cs)
                    for kc in range(2):
                        nc.sync.dma_start(
                            out=obsQ[:, kc, cs:cs + cn],
                            in_=obs_flat[c * NS + cs:c * NS + cs + cn,
                                         kc * 128:(kc + 1) * 128],
                            transpose=True,
                        )
            RQ = slice((c % 4) * NS, (c % 4) * NS + ns)
            for kc in range(2):
                nc.vector.tensor_scalar(
                    out=obsQ[:, kc, RQ], in0=obsQ[:, kc, RQ],
                    scalar1=s_sb[:, kc:kc + 1], scalar2=nms_sb[:, kc:kc + 1],
                    op0=OP.mult, op1=OP.add,
                )
            obsT8 = chpool.tile([128, 2, NS], F8, tag="obsT8")
            nc.vector.tensor_scalar(
                out=obsT8[:, :, :ns], in0=obsQ[:, :, RQ],
                scalar1=5.0, scalar2=-5.0, op0=OP.min, op1=OP.max,
            )

            # L1 (fp8 DoubleRow, K=256 in one matmul per m-chunk)
            h1 = chpool.tile([128, 8, NS], F8, tag="h1")
            for mp in range(4):
                ps = pmm.tile([128, 2, NS], F32, tag="pmm")
                for mi in range(2):
                    m = mp * 2 + mi
                    nc.tensor.matmul(
                        ps[:, mi, :ns],
                        w0_sb[:, :, m * 128:(m + 1) * 128],
                        obsT8[:, :, :ns],
                        start=True, stop=True, perf_mode=DR,
                    )
                sl = nc.scalar.activation(
                    h1[:, 2 * mp:2 * mp + 2, :ns], ps[:, :, :ns], AF.Silu)

            # L2 policy (chunks 0..31) and value (all chunks)
            h2p = chpool.tile([128, 4, NS], F8, tag="h2p")
            h2v = chpool.tile([128, 4, NS], F8, tag="h2v")
            if c < PCH:
                for mp in range(2):
                    ps = pmm.tile([128, 2, NS], F32, tag="pmm")
                    for mi in range(2):
                        m = mp * 2 + mi
                        for kb in range(2):
                            nc.tensor.matmul(
                                ps[:, mi, :ns],
                                w1p_sb[:, kb, :, m * 128:(m + 1) * 128],
                                h1[:, 2 * kb:2 * kb + 2, :ns],
                                start=(kb == 0), stop=(kb == 1), perf_mode=DR,
                            )
                    sl = nc.scalar.activation(
                        h2p[:, 2 * mp:2 * mp + 2, :ns], ps[:, :, :ns], AF.Silu)
            for mp in range(2):
                ps = pmm.tile([128, 2, NS], F32, tag="pmm")
                for mi in range(2):
                    m = mp * 2 + mi
                    for kb in range(2):
                        nc.tensor.matmul(
                            ps[:, mi, :ns],
                            w1v_sb[:, kb, :, m * 128:(m + 1) * 128],
                            h1[:, 4 + 2 * kb:4 + 2 * kb + 2, :ns],
                            start=(kb == 0), stop=(kb == 1), perf_mode=DR,
                        )
                sl = nc.scalar.activation(
                    h2v[:, 2 * mp:2 * mp + 2, :ns], ps[:, :, :ns], AF.Silu)
            last_silu = sl

            # L3 policy -> pol_fm [64, ns] fp32, then per-t transpose
            if c < PCH:
                psp = pts.tile([64, NS], F32, tag="pts")
                for kb in range(2):
                    nc.tensor.matmul(
                        psp[:, :ns], w2p_sb[:, kb, :, :],
                        h2p[:, 2 * kb:2 * kb + 2, :ns],
                        start=(kb == 0), stop=(kb == 1), perf_mode=DR,
                    )
                pol_fm = chpool.tile([64, NS], F32, tag="pol_fm")
                nc.vector.tensor_scalar(
                    out=pol_fm[:, :ns], in0=psp[:, :ns],
                    scalar1=pb2_sb, scalar2=None, op0=OP.add,
                )
                for i in range(4):
                    t = c * 4 + i
                    ptr = pts.tile([128, 64], F32, tag="pts")
                    nc.tensor.transpose(
                        ptr, pol_fm[:, i * 128:(i + 1) * 128], id_f[0:64, 0:64]
                    )
                    nc.vector.tensor_copy(pol_bt[:, t, :], ptr)

            # L3 value -> v_fm [1, ns] -> vals[:, 4c:4c+4]
            psv = pts.tile([16, NS], F32, tag="pts")
            for kb in range(2):
                nc.tensor.matmul(
                    psv[:, :ns], w2v_sb[:, kb, :, :],
                    h2v[:, 2 * kb:2 * kb + 2, :ns],
                    start=(kb == 0), stop=(kb == 1), perf_mode=DR,
                )
            v_fm = chpool.tile([1, NS], F32, tag="v_fm")
            nc.vector.tensor_scalar(
                out=v_fm[:, :ns], in0=psv[0:1, :ns],
                scalar1=vb2_sb, scalar2=None, op0=OP.add,
            )
            if ns == NS:
                vstage = chpool.tile([4, 128], F32, tag="vstage")
                nc.gpsimd.dma_start(out=vstage, in_=v_fm[0:1, :])
                pv = pts.tile([128, 4], F32, tag="pts")
                nc.tensor.transpose(pv, vstage, id_f[0:4, 0:4])
                nc.vector.tensor_copy(vals[:, 4 * c:4 * c + 4], pv)
            else:
                pv = pts.tile([128, 1], F32, tag="pts")
                nc.tensor.transpose(pv, v_fm[0:1, 0:128], id_f[0:1, 0:1])
                nc.vector.tensor_copy(vals[:, T:T + 1], pv)

        # ---- distribution tail (gated behind the last SiLU so the Exp/Ln
        # ACT chain never interleaves with SiLU LUT tables) ----
        for g in range(NG):
            t0 = g * GT
            lg = gio.tile([128, GT, 2 * ACTD], F32, tag="lg", name="lg")
            nc.sync.dma_start(
                out=lg, in_=io["lgt"][t0:t0 + GT, :, :].rearrange("t b f -> b t f"))
            ac = gio.tile([128, GT, ACTD], F32, tag="ac", name="ac")
            nc.sync.dma_start(
                out=ac, in_=io["act"][t0:t0 + GT, :, :].rearrange("t b f -> b t f"))
            ep = gio.tile([128, GT, ACTD], F32, tag="ep", name="ep")
            nc.sync.dma_start(
                out=ep, in_=io["eps"][t0:t0 + GT, :, :].rearrange("t b f -> b t f"))

            loc = pol_bt[:, t0:t0 + GT, 0:ACTD]
            sraw = pol_bt[:, t0:t0 + GT, ACTD:2 * ACTD]
            bloc = lg[:, :, 0:ACTD]
            bsraw = lg[:, :, ACTD:2 * ACTD]
            shp = [128, GT, ACTD]

            # softplus(x) = ln(exp(x)+1) via the Ln bias; 1/sigma^2 =
            # exp(-2*ln(sigma)). All within the natural_log_exp LUT set.
            sp_s = scr2.tile(shp, F32, tag="sE", name="sp_s")
            sp_b = scr2.tile(shp, F32, tag="sC", name="sp_b")
            e_s = nc.scalar.activation(sp_s, sraw, AF.Exp)
            e_b = nc.scalar.activation(sp_b, bsraw, AF.Exp)
            if last_silu is not None:
                tile.add_dep_helper(e_s.ins, last_silu.ins, sync=False,
                                    reason="dist tail after all SiLU evacs")
                tile.add_dep_helper(e_b.ins, last_silu.ins, sync=False,
                                    reason="dist tail after all SiLU evacs")
            nc.scalar.activation(sp_s, sp_s, AF.Ln, bias=1.0)   # softplus
            nc.scalar.activation(sp_b, sp_b, AF.Ln, bias=1.0)

            sg = scr1.tile(shp, F32, tag="sA", name="sg")
            nc.vector.tensor_scalar(out=sg, in0=sp_s, scalar1=0.001,
                                    scalar2=None, op0=OP.add)    # sigma
            dist = scr1.tile(shp, F32, tag="sA2", name="dist")
            nc.vector.tensor_mul(dist, sg, ep)
            nc.vector.tensor_add(dist, dist, loc)

            lsig = scr2.tile(shp, F32, tag="sB", name="lsig")
            nc.scalar.activation(lsig, sp_s, AF.Ln, bias=c_eps[:, 0:1])
            dl = scr2.tile(shp, F32, tag="sD", name="dl")
            nc.scalar.activation(dl, sp_b, AF.Ln, bias=c_eps[:, 0:1])
            rs2 = scr2.tile(shp, F32, tag="sF", name="rs2")
            nc.scalar.activation(rs2, lsig, AF.Exp, scale=-2.0)  # 1/sig^2
            nc.scalar.activation(sp_b, dl, AF.Exp, scale=-2.0)   # 1/bsig^2
            sp2 = scr1.tile(shp, F32, tag="sE2", name="sp2")
            nc.scalar.activation(sp2, dist, AF.Exp, scale=-2.0)
            nc.scalar.activation(sp2, sp2, AF.Ln, bias=1.0)      # sp(-2d)

            # the squared-deviation chains run on the otherwise-idle gpsimd
            # engine so the tail is not DVE-serial
            u = scrp.tile(shp, F32, tag="sG", name="u")
            nc.gpsimd.tensor_sub(u, ac, loc)
            nc.gpsimd.tensor_mul(u, u, u)               # (a-loc)^2
            nc.gpsimd.tensor_mul(u, u, rs2)             # u^2
            bu = scrp.tile(shp, F32, tag="sH", name="bu")
            nc.gpsimd.tensor_sub(bu, ac, bloc)
            nc.gpsimd.tensor_mul(bu, bu, bu)
            nc.gpsimd.tensor_mul(bu, bu, sp_b)          # bu^2
            nc.vector.tensor_sub(bu, bu, u)             # bu^2 - u^2
            nc.vector.tensor_sub(dl, dl, lsig)          # log bsig - log sig
            nc.vector.scalar_tensor_tensor(
                out=bu, in0=bu, scalar=0.5, in1=dl, op0=OP.mult, op1=OP.add)
            nc.vector.tensor_reduce(
                out=dsum[:, t0:t0 + GT], in_=bu, axis=AX.X, op=OP.add)

            # entropy: sum(lsig - 2*dist - 2*softplus(-2*dist)) + const
            nc.vector.scalar_tensor_tensor(
                out=sp2, in0=sp2, scalar=-2.0, in1=lsig, op0=OP.mult, op1=OP.add)
            nc.vector.scalar_tensor_tensor(
                out=sp2, in0=dist, scalar=-2.0, in1=sp2, op0=OP.mult, op1=OP.add)
            nc.vector.tensor_reduce(
                out=entsum[:, t0:t0 + GT], in_=sp2, axis=AX.X, op=OP.add)

        # ---- GAE input transposes ([t, env] -> [env, t]) ----
        def load_T(name):
            nat = gae.tile([128, 128], F32, tag=f"nat_{name}", name=f"nat_{name}")
            nc.sync.dma_start(out=nat, in_=io[name])
            ps = pts.tile([128, 128], F32, tag="pts", name=f"ps_{name}")
            nc.tensor.transpose(ps, nat, id_f)
            out = gae.tile([128, 128], F32, tag=f"bt_{name}", name=f"bt_{name}")
            nc.vector.tensor_copy(out, ps)
            return out

        rew_bt = load_T("rew")
        done_bt = load_T("don")
        trunc_bt = load_T("trn")

        # ---- GAE ([env, t] tiles) ----
        def gt(tag):
            return gae.tile([128, T], F32, tag=tag, name=tag)

        tm = gt("tm")
        nc.vector.tensor_scalar(out=tm, in0=trunc_bt, scalar1=-1.0, scalar2=1.0,
                                op0=OP.mult, op1=OP.add)          # 1 - trunc
        a1 = gt("a1")
        nc.vector.tensor_mul(a1, done_bt, tm)                     # termination
        nc.vector.tensor_scalar(out=a1, in0=a1, scalar1=-1.0, scalar2=1.0,
                                op0=OP.mult, op1=OP.add)          # 1 - term
        dl1 = gt("dl1")
        nc.vector.tensor_mul(dl1, a1, vals[:, 1:T + 1])           # (1-term)*v_tp1
        nc.vector.scalar_tensor_tensor(
            out=dl1, in0=dl1, scalar=GAMMA, in1=vals[:, 0:T], op0=OP.mult,
            op1=OP.subtract)
        nc.vector.scalar_tensor_tensor(
            out=dl1, in0=rew_bt, scalar=REW_SCALE, in1=dl1, op0=OP.mult,
            op1=OP.add)
        nc.vector.tensor_mul(dl1, dl1, tm)                        # delta
        cf = gt("cf")
        nc.vector.tensor_mul(cf, a1, tm)
        nc.vector.tensor_scalar(out=cf, in0=cf, scalar1=GAMMA * LAMBDA,
                                scalar2=None, op0=OP.mult)        # scan coeff
        sc = gt("sc")                                             # reversed vs-v
        nc.vector.tensor_tensor_scan(
            out=sc, data0=_rev(cf[:, :]), data1=_rev(dl1[:, :]),
            initial=0.0, op0=OP.mult, op1=OP.add)
        vsmv = _rev(sc[:, :])
        vs = gt("vs")
        nc.vector.tensor_add(vs, vsmv, vals[:, 0:T])
        vst = gt("vst")
        nc.vector.tensor_copy(vst[:, 0:T - 1], vs[:, 1:T])
        nc.vector.tensor_copy(vst[:, T - 1:T], vals[:, T:T + 1])
        adv = gt("adv")
        nc.vector.tensor_mul(adv, a1, vst)
        nc.vector.scalar_tensor_tensor(
            out=adv, in0=adv, scalar=GAMMA, in1=vals[:, 0:T], op0=OP.mult,
            op1=OP.subtract)
        nc.vector.scalar_tensor_tensor(
            out=adv, in0=rew_bt, scalar=REW_SCALE, in1=adv, op0=OP.mult,
            op1=OP.add)
        nc.vector.tensor_mul(adv, adv, tm)

        nc.vector.memset(sums, 0.0)
        # rho = exp(d) overflows fp32 above EXP_OVF; clamp for finite on-chip
        # math and count the ieee nan (adv==0) / -inf (adv<0) lanes so the
        # host can reinstate the exact fp32-reference semantics.
        rho = gt("rho")
        nc.vector.tensor_scalar(out=rho, in0=dsum, scalar1=80.0, scalar2=None,
                                op0=OP.min)
        nc.scalar.activation(rho, rho, AF.Exp)
        s1 = gt("s1")
        nc.vector.tensor_mul(s1, rho, adv)
        rc = gt("rc")
        nc.vector.tensor_scalar(out=rc, in0=rho, scalar1=1.0 - CLIPEPS,
                                scalar2=1.0 + CLIPEPS, op0=OP.max, op1=OP.min)
        nc.vector.tensor_mul(rc, rc, adv)
        nc.vector.tensor_tensor(out=s1, in0=s1, in1=rc, op=OP.min)
        nc.vector.tensor_reduce(out=sums[:, 0:1], in_=s1, axis=AX.X, op=OP.add)
        vsq = gt("vsq")
        nc.vector.tensor_mul(vsq, vsmv, vsmv)                     # v_err^2
        nc.vector.tensor_reduce(out=sums[:, 1:2], in_=vsq, axis=AX.X, op=OP.add)
        nc.vector.tensor_reduce(out=sums[:, 2:3], in_=entsum, axis=AX.X,
                                op=OP.add)
        gm = gt("gm")
        nc.vector.tensor_scalar(out=gm, in0=dsum, scalar1=EXP_OVF, scalar2=None,
                                op0=OP.is_gt)
        zm = gt("zm")
        nc.vector.tensor_scalar(out=zm, in0=adv, scalar1=0.0, scalar2=None,
                                op0=OP.is_equal)
        nc.vector.tensor_mul(zm, zm, gm)
        nc.vector.tensor_reduce(out=sums[:, 3:4], in_=zm, axis=AX.X, op=OP.add)
        nm = gt("nm")
        nc.vector.tensor_scalar(out=nm, in0=adv, scalar1=0.0, scalar2=None,
                                op0=OP.is_lt)
        nc.vector.tensor_mul(nm, nm, gm)
        nc.vector.tensor_reduce(out=sums[:, 4:5], in_=nm, axis=AX.X, op=OP.add)

        psf = pts.tile([8, 1], F32, tag="pts")
        nc.tensor.matmul(psf, sums, ones_sb, start=True, stop=True)
        out_sb = consts.tile([8, 1], F32)
        nc.vector.tensor_copy(out_sb, psf)
        nc.sync.dma_start(out=io["part"], in_=out_sb)


_TENSOR_SPECS = [
    ("obs", [T + 1, BL, OBS], BF16),
    ("lgt", [T, BL, 2 * ACTD], F32),
    ("act", [T, BL, ACTD], F32),
    ("eps", [T, BL, ACTD], F32),
    ("rew", [T, BL], F32),
    ("don", [T, BL], F32),
    ("trn", [T, BL], F32),
    ("w0", [2, 128, 2 * H], F8),
    ("w1p", [2, 128, 2, H], F8),
    ("w1v", [2, 128, 2, H], F8),
    ("w2p", [2, 128, 2, 2 * ACTD], F8),
    ("w2v", [2, 128, 2, 16], F8),
    ("scl", [128, 2], F32),
    ("nms", [128, 2], F32),
    ("pb2", [64, 1], F32),
    ("vb2", [1, 1], F32),
]

_NC_CACHE = None


def build_nc():
    global _NC_CACHE
    if _NC_CACHE is not None:
        return _NC_CACHE
    nc = bacc.Bacc("TRN2", target_bir_lowering=False, debug=False,
                   num_devices=NCORES)
    io = {}
    for name, shape, dt in _TENSOR_SPECS:
        io[name] = nc.dram_tensor(name, shape, dt, kind="ExternalInput").ap()
    io["part"] = nc.dram_tensor("part", [8, 1], F32, kind="ExternalOutput").ap()
    with tile.TileContext(nc) as tc:
        emit(tc, io)
    nc.compile()
    _NC_CACHE = nc
    return nc


def _dr_pack(w, kb, m):
    """[K, M] -> [kb, ki, ko, m] DoubleRow layout (k = kb*256 + ko*128 + ki)."""
    return np.ascontiguousarray(
        w.reshape(kb, 2, 128, m).transpose(0, 2, 1, 3)).astype(_F8)


def host_prep(inputs):
    """Returns (in_maps per core, combine fn)."""
    f32 = np.float32
    obs = np.asarray(inputs["observation"], f32)
    logits = np.asarray(inputs["logits"], f32)
    action = np.asarray(inputs["action"], f32)
    reward = np.asarray(inputs["reward"], f32)
    done = np.asarray(inputs["done"], f32)
    trunc = np.asarray(inputs["truncation"], f32)
    rm = np.asarray(inputs["running_mean"], f32)
    rv = np.asarray(inputs["running_variance"], f32)
    ns = float(np.asarray(inputs["num_steps"]))
    pw0 = np.asarray(inputs["pw0"], f32); pb0 = np.asarray(inputs["pb0"], f32)
    pw1 = np.asarray(inputs["pw1"], f32); pb1 = np.asarray(inputs["pb1"], f32)
    pw2 = np.asarray(inputs["pw2"], f32); pb2 = np.asarray(inputs["pb2"], f32)
    vw0 = np.asarray(inputs["vw0"], f32); vb0 = np.asarray(inputs["vb0"], f32)
    vw1 = np.asarray(inputs["vw1"], f32); vb1 = np.asarray(inputs["vb1"], f32)
    vw2 = np.asarray(inputs["vw2"], f32); vb2 = np.asarray(inputs["vb2"], f32)

    for b_ in (pb0, pb1, vb0, vb1):
        assert float(np.abs(b_).max(initial=0.0)) == 0.0, (
            "kernel assumes zero hidden-layer biases (per problem spec)")

    var = np.clip(rv / (ns + 1.0), 1e-6, 1e6)
    s = (1.0 / np.sqrt(var)).astype(f32)
    nms = (-rm * s).astype(f32)

    import jax
    import jax.numpy as jnp
    cpu = jax.devices("cpu")[0]
    with jax.default_device(cpu):
        eps = np.asarray(jax.random.normal(jax.random.key(1), (T, B, ACTD),
                                           jnp.float32))

    w0c = np.concatenate([pw0, vw0], axis=1)          # [256, 1024]
    w0 = np.ascontiguousarray(w0c.reshape(2, 128, 2 * H)).astype(_F8)
    w1p = _dr_pack(pw1, 2, H)
    w1v = _dr_pack(vw1, 2, H)
    w2p = _dr_pack(pw2, 2, 2 * ACTD)
    vw2_pad = np.zeros((H, 16), f32)
    vw2_pad[:, 0:1] = vw2
    w2v = _dr_pack(vw2_pad, 2, 16)
    scl = np.ascontiguousarray(s.reshape(2, 128).T)
    nmsr = np.ascontiguousarray(nms.reshape(2, 128).T)
    pb2r = np.ascontiguousarray(pb2.reshape(64, 1))
    vb2r = np.ascontiguousarray(vb2.reshape(1, 1))
    obs_bf = obs.astype(_BF)

    in_maps = []
    for c in range(NCORES):
        bs = slice(c * BL, (c + 1) * BL)
        in_maps.append(dict(
            obs=np.ascontiguousarray(obs_bf[:, bs, :]),
            lgt=np.ascontiguousarray(logits[:, bs, :]),
            act=np.ascontiguousarray(action[:, bs, :]),
            eps=np.ascontiguousarray(eps[:, bs, :]),
            rew=np.ascontiguousarray(reward[:, bs]),
            don=np.ascontiguousarray(done[:, bs]),
            trn=np.ascontiguousarray(trunc[:, bs]),
            w0=w0, w1p=w1p, w1v=w1v, w2p=w2p, w2v=w2v,
            scl=scl, nms=nmsr, pb2=pb2r, vb2=vb2r,
        ))

    def combine(parts):
        tot = np.zeros(5, np.float64)
        for p in parts:
            tot += np.asarray(p, np.float64)[0:5, 0].ravel()
        n = float(T * B)
        ms, ve, es, nan_c, ninf_c = tot
        # Reinstate ieee fp32 semantics of the reference: surr1 = inf*adv
        # lanes produce nan (adv==0) or -inf (adv<0) and dominate the mean.
        if nan_c > 0:
            ms = np.nan
        elif ninf_c > 0:
            ms = -np.inf
        loss = (-ms / n) + 0.25 * (ve / n) - ENT_COST * (es / n + ENT_CONST)
        return np.float32(loss)

    return in_maps, combine


def run_sharded(inputs, **kw):
    nc = build_nc()
    in_maps, combine = host_prep(inputs)
    res = bass_utils.run_bass_kernel_spmd(
        nc, in_maps, core_ids=list(range(NCORES)), **kw)
    parts = [r["part"] for r in res.results]
    return combine(parts), res


def kernel(**inputs):
    out, _ = run_sharded(inputs)
    return out


# revision 42
# speedup vs baseline: 1.2241x; 1.2241x over previous
"""PPO-style loss kernel for Trainium2, data-parallel over the env axis B.

Contract: kernel(**inputs) takes the full unsharded inputs (numpy or jax
arrays) keyed as in setup_inputs() and returns the full scalar loss
(np.float32). Internally shards B=1024 across 8 NeuronCores (128 envs
each), runs one SPMD Bass program per core, and combines the per-core
partial sums on the host.

Structure (per core, 128 envs):
  - obs [129,128,256] arrives bf16, is DMA-xbar-transposed per row-chunk
    into feature-major tiles, normalized ((x*s+b) then clip +-5) on DVE and
    cast to fp8e4m3.
  - Both MLPs run feature-major in fp8 with DoubleRow matmuls (2 k-rows per
    PE cell); L1 computes policy+value layer-1 in one pass over a fused
    [256,1024] weight. SiLU + psum evacuation fused on the scalar engine.
  - Policy logits are PE-transposed per timestep into an [env, t, 64]
    resident tile; values land via small PE transposes into [env, t+1].
  - The distribution math (tanh-Normal log-probs, entropy) runs batch-major
    as a tail gated behind the last SiLU evacuation so its Exp/Ln chain
    never interleaves with SiLU (different ACT LUT tables; each interleave
    costs two 1.28us table loads).
  - GAE is one DVE tensor_tensor_scan over reversed [env, t] tiles.
  - rho = exp(target_lp - behaviour_lp): the tanh log-det-jacobian and
    0.5*log(2pi) terms cancel and are never computed. exp overflow lanes
    (d > 88.72, where fp32 gives inf and the reference loss goes nan via
    inf*0) are counted on-chip and the ieee result is reinstated on host.
"""

import os
import sys

for _p in ("/opt/trn_rl_repo", "/root/.axon_site/_ro/trn_rl_repo"):
    if os.path.isdir(_p) and _p not in sys.path:
        sys.path.insert(0, _p)

import numpy as np
import ml_dtypes

import concourse.bass as bass
import concourse.tile as tile
from concourse import bacc, mybir
from concourse import bass_utils
from concourse.masks import make_identity

F32 = mybir.dt.float32
BF16 = mybir.dt.bfloat16
F8 = mybir.dt.float8e4
AX = mybir.AxisListType
OP = mybir.AluOpType
AF = mybir.ActivationFunctionType
DR = mybir.MatmulPerfMode.DoubleRow

T, B, OBS, ACTD, H = 128, 1024, 256, 32, 512
NCORES = 8
BL = B // NCORES                 # 128 envs per core
ROWS = (T + 1) * BL              # 16512 rows through the value net
PROWS = T * BL                   # 16384 rows through the policy net
NS = 512                         # row-chunk size
NCH = (ROWS + NS - 1) // NS      # 33 chunks (last one has 128 rows)
PCH = PROWS // NS                # 32 policy chunks
GT = 32                          # timesteps per distribution group
NG = T // GT                     # 4 groups

GAMMA = 0.97
LAMBDA = 0.95
CLIPEPS = 0.3
REW_SCALE = 0.1
ENT_COST = 0.01
LOG_2PI_HALF = 0.5 * float(np.log(2.0 * np.pi))
LOG2 = float(np.log(2.0))
ENT_CONST = ACTD * (0.5 + LOG_2PI_HALF + 2.0 * LOG2)
EXP_OVF = 88.72283                     # exp(x) == inf in fp32 above this

_BF = ml_dtypes.bfloat16
_F8 = mybir.dt.np(F8)


def _rev(ap):
    """Reverse the (single) free dim of a 2D [P, F] AP view."""
    (ps, pc), (fs, fc) = ap.ap[0], ap.ap[1]
    return bass.AP(
        tensor=ap.tensor,
        offset=ap.offset + (fc - 1) * fs,
        ap=[[ps, pc], [-fs, fc]],
    )


def emit(tc, io):
    nc = tc.nc
    from contextlib import ExitStack

    ctx = ExitStack()
    with ctx:
        consts = ctx.enter_context(tc.tile_pool(name="consts", bufs=1))
        chpool = ctx.enter_context(tc.tile_pool(name="ch", bufs=2))
        gio = ctx.enter_context(tc.tile_pool(name="gio", bufs=2))
        scr2 = ctx.enter_context(tc.tile_pool(name="scr2", bufs=2))
        scr1 = ctx.enter_context(tc.tile_pool(name="scr1", bufs=1))
        scrp = ctx.enter_context(tc.tile_pool(name="scrp", bufs=2))
        gae = ctx.enter_context(tc.tile_pool(name="gae", bufs=1))
        pmm = ctx.enter_context(tc.tile_pool(name="pmm", bufs=3, space="PSUM"))
        pts = ctx.enter_context(tc.tile_pool(name="pts", bufs=2, space="PSUM"))

        # first obs piece ahead of the weight DMAs on the SP queue: its
        # 6.6us xbar transfer heads the chunk-0 critical path
        QR = 4 * NS                                      # xbar piece: 4 chunks
        obs_flat = io["obs"].flatten_outer_dims()        # [16512, 256]
        obsQ0 = chpool.tile([128, 2, QR], BF16, tag="obsQ", name="obsQ0")
        for kc in range(2):
            nc.sync.dma_start(
                out=obsQ0[:, kc, :],
                in_=obs_flat[0:QR, kc * 128:(kc + 1) * 128],
                transpose=True,
            )

        # ---- constants / weights into SBUF ----
        w0_sb = consts.tile([128, 2, 2 * H], F8)          # [ki, ko, m]
        nc.sync.dma_start(out=w0_sb, in_=io["w0"].rearrange("ko ki m -> ki ko m"))
        w1p_sb = consts.tile([128, 2, 2, H], F8)          # [ki, kb, ko, m]
        nc.sync.dma_start(out=w1p_sb,
                          in_=io["w1p"].rearrange("kb ki ko m -> ki kb ko m"))
        w1v_sb = consts.tile([128, 2, 2, H], F8)
        nc.sync.dma_start(out=w1v_sb,
                          in_=io["w1v"].rearrange("kb ki ko m -> ki kb ko m"))
        w2p_sb = consts.tile([128, 2, 2, 2 * ACTD], F8)
        nc.sync.dma_start(out=w2p_sb,
                          in_=io["w2p"].rearrange("kb ki ko m -> ki kb ko m"))
        # value head padded to M=16: DoubleRow ldweights needs the k-pair
        # stride (= M elements) to be 16-byte aligned
        w2v_sb = consts.tile([128, 2, 2, 16], F8)
        nc.sync.dma_start(out=w2v_sb,
                          in_=io["w2v"].rearrange("kb ki ko m -> ki kb ko m"))
        s_sb = consts.tile([128, 2], F32)
        nc.sync.dma_start(out=s_sb, in_=io["scl"])
        nms_sb = consts.tile([128, 2], F32)
        nc.sync.dma_start(out=nms_sb, in_=io["nms"])
        pb2_sb = consts.tile([64, 1], F32)
        nc.sync.dma_start(out=pb2_sb, in_=io["pb2"])
        vb2_sb = consts.tile([1, 1], F32)
        nc.sync.dma_start(out=vb2_sb, in_=io["vb2"])

        id_f = consts.tile([128, 128], F32)
        make_identity(nc, id_f)
        ones_sb = consts.tile([128, 1], F32)
        nc.vector.memset(ones_sb, 1.0)
        c_eps = consts.tile([128, 1], F32)          # 0.001 bias for Ln(sp+eps)
        nc.vector.memset(c_eps, 0.001)

        # persistent accumulators ([env, ...] layouts)
        pol_bt = consts.tile([128, T, 2 * ACTD], F32)    # policy logits
        vals = consts.tile([128, T + 1], F32)            # values + bootstrap
        dsum = consts.tile([128, T], F32)                # target_lp - behaviour_lp
        entsum = consts.tile([128, T], F32)              # entropy partials
        sums = consts.tile([128, 8], F32)

        last_silu = None

        # ---- main row-chunk loop ----
        for c in range(NCH):
            ns = min(NS, ROWS - c * NS)
            R = slice(c * NS, c * NS + ns)

            # obs: xbar-transpose to feature-major bf16 in 2048-row pieces,
            # then per chunk z = clip(x*s + nms, +-5) and cast to fp8 on DVE
            if c % 4 == 0:
                if c == 0:
                    obsQ = obsQ0
                else:
                    qr = min(QR, ROWS - c * NS)
                    obsQ = chpool.tile([128, 2, QR], BF16, tag="obsQ")
                    for kc in range(2):
                        nc.sync.dma_start(
                            out=obsQ[:, kc, :qr],
                            in_=obs_flat[c * NS:c * NS + qr,
                                         kc * 128:(kc + 1) * 128],
                            transpose=True,
                        )
            RQ = slice((c % 4) * NS, (c % 4) * NS + ns)
            for kc in range(2):
                nc.vector.tensor_scalar(
                    out=obsQ[:, kc, RQ], in0=obsQ[:, kc, RQ],
                    scalar1=s_sb[:, kc:kc + 1], scalar2=nms_sb[:, kc:kc + 1],
                    op0=OP.mult, op1=OP.add,
                )
            obsT8 = chpool.tile([128, 2, NS], F8, tag="obsT8")
            nc.vector.tensor_scalar(
                out=obsT8[:, :, :ns], in0=obsQ[:, :, RQ],
                scalar1=5.0, scalar2=-5.0, op0=OP.min, op1=OP.max,
            )

            # L1 (fp8 DoubleRow, K=256 in one matmul per m-chunk)
            h1 = chpool.tile([128, 8, NS], F8, tag="h1")
            for mp in range(4):
                ps = pmm.tile([128, 2, NS], F32, tag="pmm")
                for mi in range(2):
                    m = mp * 2 + mi
                    nc.tensor.matmul(
                        ps[:, mi, :ns],
                        w0_sb[:, :, m * 128:(m + 1) * 128],
                        obsT8[:, :, :ns],
                        start=True, stop=True, perf_mode=DR,
                    )
                sl = nc.scalar.activation(
                    h1[:, 2 * mp:2 * mp + 2, :ns], ps[:, :, :ns], AF.Silu)

            # L2 policy (chunks 0..31) and value (all chunks)
            h2p = chpool.tile([128, 4, NS], F8, tag="h2p")
            h2v = chpool.tile([128, 4, NS], F8, tag="h2v")
            if c < PCH:
                for mp in range(2):
                    ps = pmm.tile([128, 2, NS], F32, tag="pmm")
                    for mi in range(2):
                        m = mp * 2 + mi
                        for kb in range(2):
                            nc.tensor.matmul(
                                ps[:, mi, :ns],
                                w1p_sb[:, kb, :, m * 128:(m + 1) * 128],
                                h1[:, 2 * kb:2 * kb + 2, :ns],
                                start=(kb == 0), stop=(kb == 1), perf_mode=DR,
                            )
                    sl = nc.scalar.activation(
                        h2p[:, 2 * mp:2 * mp + 2, :ns], ps[:, :, :ns], AF.Silu)
            for mp in range(2):
                ps = pmm.tile([128, 2, NS], F32, tag="pmm")
                for mi in range(2):
                    m = mp * 2 + mi
                    for kb in range(2):
                        nc.tensor.matmul(
                            ps[:, mi, :ns],
                            w1v_sb[:, kb, :, m * 128:(m + 1) * 128],
                            h1[:, 4 + 2 * kb:4 + 2 * kb + 2, :ns],
                            start=(kb == 0), stop=(kb == 1), perf_mode=DR,
                        )
                sl = nc.scalar.activation(
                    h2v[:, 2 * mp:2 * mp + 2, :ns], ps[:, :, :ns], AF.Silu)
            last_silu = sl

            # L3 policy -> pol_fm [64, ns] fp32, then per-t transpose
            if c < PCH:
                psp = pts.tile([64, NS], F32, tag="pts")
                for kb in range(2):
                    nc.tensor.matmul(
                        psp[:, :ns], w2p_sb[:, kb, :, :],
                        h2p[:, 2 * kb:2 * kb + 2, :ns],
                        start=(kb == 0), stop=(kb == 1), perf_mode=DR,
                    )
                pol_fm = chpool.tile([64, NS], F32, tag="pol_fm")
                nc.vector.tensor_scalar(
                    out=pol_fm[:, :ns], in0=psp[:, :ns],
                    scalar1=pb2_sb, scalar2=None, op0=OP.add,
                )
                for i in range(4):
                    t = c * 4 + i
                    ptr = pts.tile([128, 64], F32, tag="pts")
                    nc.tensor.transpose(
                        ptr, pol_fm[:, i * 128:(i + 1) * 128], id_f[0:64, 0:64]
                    )
                    nc.vector.tensor_copy(pol_bt[:, t, :], ptr)

            # L3 value -> v_fm [1, ns] -> vals[:, 4c:4c+4]
            psv = pts.tile([16, NS], F32, tag="pts")
            for kb in range(2):
                nc.tensor.matmul(
                    psv[:, :ns], w2v_sb[:, kb, :, :],
                    h2v[:, 2 * kb:2 * kb + 2, :ns],
                    start=(kb == 0), stop=(kb == 1), perf_mode=DR,
                )
            v_fm = chpool.tile([1, NS], F32, tag="v_fm")
            nc.vector.tensor_scalar(
                out=v_fm[:, :ns], in0=psv[0:1, :ns],
                scalar1=vb2_sb, scalar2=None, op0=OP.add,
            )
            if ns == NS:
                vstage = chpool.tile([4, 128], F32, tag="vstage")
                nc.gpsimd.dma_start(out=vstage, in_=v_fm[0:1, :])
                pv = pts.tile([128, 4], F32, tag="pts")
                nc.tensor.transpose(pv, vstage, id_f[0:4, 0:4])
                nc.vector.tensor_copy(vals[:, 4 * c:4 * c + 4], pv)
            else:
                pv = pts.tile([128, 1], F32, tag="pts")
                nc.tensor.transpose(pv, v_fm[0:1, 0:128], id_f[0:1, 0:1])
                nc.vector.tensor_copy(vals[:, T:T + 1], pv)

        # ---- distribution tail (gated behind the last SiLU so the Exp/Ln
        # ACT chain never interleaves with SiLU LUT tables) ----
        for g in range(NG):
            t0 = g * GT
            lg = gio.tile([128, GT, 2 * ACTD], F32, tag="lg", name="lg")
            nc.sync.dma_start(
                out=lg, in_=io["lgt"][t0:t0 + GT, :, :].rearrange("t b f -> b t f"))
            ac = gio.tile([128, GT, ACTD], F32, tag="ac", name="ac")
            nc.sync.dma_start(
                out=ac, in_=io["act"][t0:t0 + GT, :, :].rearrange("t b f -> b t f"))
            ep = gio.tile([128, GT, ACTD], F32, tag="ep", name="ep")
            nc.sync.dma_start(
                out=ep, in_=io["eps"][t0:t0 + GT, :, :].rearrange("t b f -> b t f"))

            loc = pol_bt[:, t0:t0 + GT, 0:ACTD]
            sraw = pol_bt[:, t0:t0 + GT, ACTD:2 * ACTD]
            bloc = lg[:, :, 0:ACTD]
            bsraw = lg[:, :, ACTD:2 * ACTD]
            shp = [128, GT, ACTD]

            # softplus(x) = ln(exp(x)+1) via the Ln bias; 1/sigma^2 =
            # exp(-2*ln(sigma)). All within the natural_log_exp LUT set.
            sp_s = scr2.tile(shp, F32, tag="sE", name="sp_s")
            sp_b = scr2.tile(shp, F32, tag="sC", name="sp_b")
            e_s = nc.scalar.activation(sp_s, sraw, AF.Exp)
            e_b = nc.scalar.activation(sp_b, bsraw, AF.Exp)
            if last_silu is not None:
                tile.add_dep_helper(e_s.ins, last_silu.ins, sync=False,
                                    reason="dist tail after all SiLU evacs")
                tile.add_dep_helper(e_b.ins, last_silu.ins, sync=False,
                                    reason="dist tail after all SiLU evacs")
            nc.scalar.activation(sp_s, sp_s, AF.Ln, bias=1.0)   # softplus
            nc.scalar.activation(sp_b, sp_b, AF.Ln, bias=1.0)

            sg = scr1.tile(shp, F32, tag="sA", name="sg")
            nc.vector.tensor_scalar(out=sg, in0=sp_s, scalar1=0.001,
                                    scalar2=None, op0=OP.add)    # sigma
            dist = scr1.tile(shp, F32, tag="sA2", name="dist")
            nc.vector.tensor_mul(dist, sg, ep)
            nc.vector.tensor_add(dist, dist, loc)

            lsig = scr2.tile(shp, F32, tag="sB", name="lsig")
            nc.scalar.activation(lsig, sp_s, AF.Ln, bias=c_eps[:, 0:1])
            dl = scr2.tile(shp, F32, tag="sD", name="dl")
            nc.scalar.activation(dl, sp_b, AF.Ln, bias=c_eps[:, 0:1])
            rs2 = scr2.tile(shp, F32, tag="sF", name="rs2")
            nc.scalar.activation(rs2, lsig, AF.Exp, scale=-2.0)  # 1/sig^2
            nc.scalar.activation(sp_b, dl, AF.Exp, scale=-2.0)   # 1/bsig^2
            sp2 = scr1.tile(shp, F32, tag="sE2", name="sp2")
            nc.scalar.activation(sp2, dist, AF.Exp, scale=-2.0)
            nc.scalar.activation(sp2, sp2, AF.Ln, bias=1.0)      # sp(-2d)

            # the squared-deviation chains run on the otherwise-idle gpsimd
            # engine so the tail is not DVE-serial
            u = scrp.tile(shp, F32, tag="sG", name="u")
            nc.gpsimd.tensor_sub(u, ac, loc)
            nc.gpsimd.tensor_mul(u, u, u)               # (a-loc)^2
            nc.gpsimd.tensor_mul(u, u, rs2)             # u^2
            bu = scrp.tile(shp, F32, tag="sH", name="bu")
            nc.gpsimd.tensor_sub(bu, ac, bloc)
            nc.gpsimd.tensor_mul(bu, bu, bu)
            nc.gpsimd.tensor_mul(bu, bu, sp_b)          # bu^2
            nc.vector.tensor_sub(bu, bu, u)             # bu^2 - u^2
            nc.vector.tensor_sub(dl, dl, lsig)          # log bsig - log sig
            nc.vector.scalar_tensor_tensor(
                out=bu, in0=bu, scalar=0.5, in1=dl, op0=OP.mult, op1=OP.add)
            nc.vector.tensor_reduce(
                out=dsum[:, t0:t0 + GT], in_=bu, axis=AX.X, op=OP.add)

            # entropy: sum(lsig - 2*dist - 2*softplus(-2*dist)) + const
            nc.vector.scalar_tensor_tensor(
                out=sp2, in0=sp2, scalar=-2.0, in1=lsig, op0=OP.mult, op1=OP.add)
            nc.vector.scalar_tensor_tensor(
                out=sp2, in0=dist, scalar=-2.0, in1=sp2, op0=OP.mult, op1=OP.add)
            nc.vector.tensor_reduce(
                out=entsum[:, t0:t0 + GT], in_=sp2, axis=AX.X, op=OP.add)

        # ---- GAE input transposes ([t, env] -> [env, t]) ----
        def load_T(name):
            nat = gae.tile([128, 128], F32, tag=f"nat_{name}", name=f"nat_{name}")
            nc.sync.dma_start(out=nat, in_=io[name])
            ps = pts.tile([128, 128], F32, tag="pts", name=f"ps_{name}")
            nc.tensor.transpose(ps, nat, id_f)
            out = gae.tile([128, 128], F32, tag=f"bt_{name}", name=f"bt_{name}")
            nc.vector.tensor_copy(out, ps)
            return out

        rew_bt = load_T("rew")
        done_bt = load_T("don")
        trunc_bt = load_T("trn")

        # ---- GAE ([env, t] tiles) ----
        def gt(tag):
            return gae.tile([128, T], F32, tag=tag, name=tag)

        tm = gt("tm")
        nc.vector.tensor_scalar(out=tm, in0=trunc_bt, scalar1=-1.0, scalar2=1.0,
                                op0=OP.mult, op1=OP.add)          # 1 - trunc
        a1 = gt("a1")
        nc.vector.tensor_mul(a1, done_bt, tm)                     # termination
        nc.vector.tensor_scalar(out=a1, in0=a1, scalar1=-1.0, scalar2=1.0,
                                op0=OP.mult, op1=OP.add)          # 1 - term
        dl1 = gt("dl1")
        nc.vector.tensor_mul(dl1, a1, vals[:, 1:T + 1])           # (1-term)*v_tp1
        nc.vector.scalar_tensor_tensor(
            out=dl1, in0=dl1, scalar=GAMMA, in1=vals[:, 0:T], op0=OP.mult,
            op1=OP.subtract)
        nc.vector.scalar_tensor_tensor(
            out=dl1, in0=rew_bt, scalar=REW_SCALE, in1=dl1, op0=OP.mult,
            op1=OP.add)
        nc.vector.tensor_mul(dl1, dl1, tm)                        # delta
        cf = gt("cf")
        nc.vector.tensor_mul(cf, a1, tm)
        nc.vector.tensor_scalar(out=cf, in0=cf, scalar1=GAMMA * LAMBDA,
                                scalar2=None, op0=OP.mult)        # scan coeff
        sc = gt("sc")                                             # reversed vs-v
        nc.vector.tensor_tensor_scan(
            out=sc, data0=_rev(cf[:, :]), data1=_rev(dl1[:, :]),
            initial=0.0, op0=OP.mult, op1=OP.add)
        vsmv = _rev(sc[:, :])
        vs = gt("vs")
        nc.vector.tensor_add(vs, vsmv, vals[:, 0:T])
        vst = gt("vst")
        nc.vector.tensor_copy(vst[:, 0:T - 1], vs[:, 1:T])
        nc.vector.tensor_copy(vst[:, T - 1:T], vals[:, T:T + 1])
        adv = gt("adv")
        nc.vector.tensor_mul(adv, a1, vst)
        nc.vector.scalar_tensor_tensor(
            out=adv, in0=adv, scalar=GAMMA, in1=vals[:, 0:T], op0=OP.mult,
            op1=OP.subtract)
        nc.vector.scalar_tensor_tensor(
            out=adv, in0=rew_bt, scalar=REW_SCALE, in1=adv, op0=OP.mult,
            op1=OP.add)
        nc.vector.tensor_mul(adv, adv, tm)

        nc.vector.memset(sums, 0.0)
        # rho = exp(d) overflows fp32 above EXP_OVF; clamp for finite on-chip
        # math and count the ieee nan (adv==0) / -inf (adv<0) lanes so the
        # host can reinstate the exact fp32-reference semantics.
        rho = gt("rho")
        nc.vector.tensor_scalar(out=rho, in0=dsum, scalar1=80.0, scalar2=None,
                                op0=OP.min)
        nc.scalar.activation(rho, rho, AF.Exp)
        s1 = gt("s1")
        nc.vector.tensor_mul(s1, rho, adv)
        rc = gt("rc")
        nc.vector.tensor_scalar(out=rc, in0=rho, scalar1=1.0 - CLIPEPS,
                                scalar2=1.0 + CLIPEPS, op0=OP.max, op1=OP.min)
        nc.vector.tensor_mul(rc, rc, adv)
        nc.vector.tensor_tensor(out=s1, in0=s1, in1=rc, op=OP.min)
        nc.vector.tensor_reduce(out=sums[:, 0:1], in_=s1, axis=AX.X, op=OP.add)
        vsq = gt("vsq")
        nc.vector.tensor_mul(vsq, vsmv, vsmv)                     # v_err^2
        nc.vector.tensor_reduce(out=sums[:, 1:2], in_=vsq, axis=AX.X, op=OP.add)
        nc.vector.tensor_reduce(out=sums[:, 2:3], in_=entsum, axis=AX.X,
                                op=OP.add)
        gm = gt("gm")
        nc.vector.tensor_scalar(out=gm, in0=dsum, scalar1=EXP_OVF, scalar2=None,
                                op0=OP.is_gt)
        zm = gt("zm")
        nc.vector.tensor_scalar(out=zm, in0=adv, scalar1=0.0, scalar2=None,
                                op0=OP.is_equal)
        nc.vector.tensor_mul(zm, zm, gm)
        nc.vector.tensor_reduce(out=sums[:, 3:4], in_=zm, axis=AX.X, op=OP.add)
        nm = gt("nm")
        nc.vector.tensor_scalar(out=nm, in0=adv, scalar1=0.0, scalar2=None,
                                op0=OP.is_lt)
        nc.vector.tensor_mul(nm, nm, gm)
        nc.vector.tensor_reduce(out=sums[:, 4:5], in_=nm, axis=AX.X, op=OP.add)

        psf = pts.tile([8, 1], F32, tag="pts")
        nc.tensor.matmul(psf, sums, ones_sb, start=True, stop=True)
        out_sb = consts.tile([8, 1], F32)
        nc.vector.tensor_copy(out_sb, psf)
        nc.sync.dma_start(out=io["part"], in_=out_sb)


_TENSOR_SPECS = [
    ("obs", [T + 1, BL, OBS], BF16),
    ("lgt", [T, BL, 2 * ACTD], F32),
    ("act", [T, BL, ACTD], F32),
    ("eps", [T, BL, ACTD], F32),
    ("rew", [T, BL], F32),
    ("don", [T, BL], F32),
    ("trn", [T, BL], F32),
    ("w0", [2, 128, 2 * H], F8),
    ("w1p", [2, 128, 2, H], F8),
    ("w1v", [2, 128, 2, H], F8),
    ("w2p", [2, 128, 2, 2 * ACTD], F8),
    ("w2v", [2, 128, 2, 16], F8),
    ("scl", [128, 2], F32),
    ("nms", [128, 2], F32),
    ("pb2", [64, 1], F32),
    ("vb2", [1, 1], F32),
]

_NC_CACHE = None


def build_nc():
    global _NC_CACHE
    if _NC_CACHE is not None:
        return _NC_CACHE
    nc = bacc.Bacc("TRN2", target_bir_lowering=False, debug=False,
                   num_devices=NCORES)
    io = {}
    for name, shape, dt in _TENSOR_SPECS:
        io[name] = nc.dram_tensor(name, shape, dt, kind="ExternalInput").ap()
    io["part"] = nc.dram_tensor("part", [8, 1], F32, kind="ExternalOutput").ap()
    with tile.TileContext(nc) as tc:
        emit(tc, io)
    nc.compile()
    _NC_CACHE = nc
    return nc


def _dr_pack(w, kb, m):
    """[K, M] -> [kb, ki, ko, m] DoubleRow layout (k = kb*256 + ko*128 + ki)."""
    return np.ascontiguousarray(
        w.reshape(kb, 2, 128, m).transpose(0, 2, 1, 3)).astype(_F8)


def host_prep(inputs):
    """Returns (in_maps per core, combine fn)."""
    f32 = np.float32
    obs = np.asarray(inputs["observation"], f32)
    logits = np.asarray(inputs["logits"], f32)
    action = np.asarray(inputs["action"], f32)
    reward = np.asarray(inputs["reward"], f32)
    done = np.asarray(inputs["done"], f32)
    trunc = np.asarray(inputs["truncation"], f32)
    rm = np.asarray(inputs["running_mean"], f32)
    rv = np.asarray(inputs["running_variance"], f32)
    ns = float(np.asarray(inputs["num_steps"]))
    pw0 = np.asarray(inputs["pw0"], f32); pb0 = np.asarray(inputs["pb0"], f32)
    pw1 = np.asarray(inputs["pw1"], f32); pb1 = np.asarray(inputs["pb1"], f32)
    pw2 = np.asarray(inputs["pw2"], f32); pb2 = np.asarray(inputs["pb2"], f32)
    vw0 = np.asarray(inputs["vw0"], f32); vb0 = np.asarray(inputs["vb0"], f32)
    vw1 = np.asarray(inputs["vw1"], f32); vb1 = np.asarray(inputs["vb1"], f32)
    vw2 = np.asarray(inputs["vw2"], f32); vb2 = np.asarray(inputs["vb2"], f32)

    for b_ in (pb0, pb1, vb0, vb1):
        assert float(np.abs(b_).max(initial=0.0)) == 0.0, (
            "kernel assumes zero hidden-layer biases (per problem spec)")

    var = np.clip(rv / (ns + 1.0), 1e-6, 1e6)
    s = (1.0 / np.sqrt(var)).astype(f32)
    nms = (-rm * s).astype(f32)

    import jax
    import jax.numpy as jnp
    cpu = jax.devices("cpu")[0]
    with jax.default_device(cpu):
        eps = np.asarray(jax.random.normal(jax.random.key(1), (T, B, ACTD),
                                           jnp.float32))

    w0c = np.concatenate([pw0, vw0], axis=1)          # [256, 1024]
    w0 = np.ascontiguousarray(w0c.reshape(2, 128, 2 * H)).astype(_F8)
    w1p = _dr_pack(pw1, 2, H)
    w1v = _dr_pack(vw1, 2, H)
    w2p = _dr_pack(pw2, 2, 2 * ACTD)
    vw2_pad = np.zeros((H, 16), f32)
    vw2_pad[:, 0:1] = vw2
    w2v = _dr_pack(vw2_pad, 2, 16)
    scl = np.ascontiguousarray(s.reshape(2, 128).T)
    nmsr = np.ascontiguousarray(nms.reshape(2, 128).T)
    pb2r = np.ascontiguousarray(pb2.reshape(64, 1))
    vb2r = np.ascontiguousarray(vb2.reshape(1, 1))
    obs_bf = obs.astype(_BF)

    in_maps = []
    for c in range(NCORES):
        bs = slice(c * BL, (c + 1) * BL)
        in_maps.append(dict(
            obs=np.ascontiguousarray(obs_bf[:, bs, :]),
            lgt=np.ascontiguousarray(logits[:, bs, :]),
            act=np.ascontiguousarray(action[:, bs, :]),
            eps=np.ascontiguousarray(eps[:, bs, :]),
            rew=np.ascontiguousarray(reward[:, bs]),
            don=np.ascontiguousarray(done[:, bs]),
            trn=np.ascontiguousarray(trunc[:, bs]),
            w0=w0, w1p=w1p, w1v=w1v, w2p=w2p, w2v=w2v,
            scl=scl, nms=nmsr, pb2=pb2r, vb2=vb2r,
        ))

    def combine(parts):
        tot = np.zeros(5, np.float64)
        for p in parts:
            tot += np.asarray(p, np.float64)[0:5, 0].ravel()
        n = float(T * B)
        ms, ve, es, nan_c, ninf_c = tot
        # Reinstate ieee fp32 semantics of the reference: surr1 = inf*adv
        # lanes produce nan (adv==0) or -inf (adv<0) and dominate the mean.
        if nan_c > 0:
            ms = np.nan
        elif ninf_c > 0:
            ms = -np.inf
        loss = (-ms / n) + 0.25 * (ve / n) - ENT_COST * (es / n + ENT_CONST)
        return np.float32(loss)

    return in_maps, combine


def run_sharded(inputs, **kw):
    nc = build_nc()
    in_maps, combine = host_prep(inputs)
    res = bass_utils.run_bass_kernel_spmd(
        nc, in_maps, core_ids=list(range(NCORES)), **kw)
    parts = [r["part"] for r in res.results]
    return combine(parts), res


def kernel(**inputs):
    out, _ = run_sharded(inputs)
    return out
